# revision 5
# baseline (speedup 1.0000x reference)
"""Trainium2 Bass kernel for ConditionalCrossAttention (DETR-style).

Shapes (hardcoded): query/query_pos/query_sine_embed [300,16,256],
key/key_pos/value [4096,16,256], 7 linear projections 256x256, H=8 heads,
per-head dim 32, concat content|positional -> head dim 64, softmax over
L=4096, output projection + residual.

Sharding: batch B=16 across 8 cores (2 per core), no collectives.

Device-side design per core (b in {0,1}, head pairs g in {0..3}):
  - Host pre-transposes inputs to [C, tokens] bf16 and builds merged
    "concat layout" projection weights (scale folded into q side).
  - DMA order: weights, q-side, then key/kpos/val in 1024-column chunks
    so the first kproj starts ~9us in; batch b+1 inputs stream during
    batch b compute.
  - qcombT [4][128,300], kcombT per (g,l4) [128,512] via PE matmuls,
    bias applied during DVE eviction.
  - vext pre-pass interleaved into pair-0's attention loop: value
    projection in layout [tok, 264]: per head h 33 cols = [v_h(32)|ones].
  - scores^T [l,n]: two K=64 matmuls per pair (row tiles (0,0)/(64,0),
    concurrent), Exp on ScalarE over both banks -> bf16 SBUF.
  - AV: two col-tiled matmuls per (pair, l-tile): M=33 ([v|ones]) at
    tile positions (0,0) and (0,64) -> ONE psum bank per pair
    (A rows 0:33, B rows 64:97; denominators at rows 32/96).
    avps bufs=2 so pair g+1 accumulates while pair g normalizes.
  - normalize: DVE reciprocal straight from PSUM rows 32/96, PE
    ones-matmul replicates recip across partitions, DVE multiply ->
    attnT bf16 rows {0:32, 64:96}.
  - o-proj: K=32 chunks from attn rows 0:32 (psum A) and 64:96 (psum B)
    in parallel row tiles, b_o folded into the fp32 residual host-side,
    two DVE adds, DMA out fp32.
"""

import sys

for _p in ("/opt/trn_rl_repo", "/root/.axon_site/_ro/trn_rl_repo"):
    if _p not in sys.path:
        sys.path.insert(0, _p)

import numpy as np
import ml_dtypes

BF16 = ml_dtypes.bfloat16

N_Q = 300
HW = 4096
B = 16
C = 256
H = 8
DH = 32
NCORES = 8
BPC = B // NCORES  # batches per core
SCALE = (2 * DH) ** (-0.5)

_COMPILED = {}


def _patch_tile_tail_drain():
    """walrus in this container rejects >1-2 sync waits on the Tile tail
    Drain; split them across a chain of single-wait drains."""
    import concourse.mybir as mybir
    from concourse.tile import TileContext
    from concourse.vector_clock import ScopedClock

    if getattr(TileContext, "_ant_drain_patched", False):
        return

    def _patched(self, tick_clock, wait_clock):
        nc = self.nc
        drain_inst = nc.sync.drain()
        wait_clock.add_sem_waits(
            drain_inst.ins, ScopedClock({None: tick_clock.global_clock})
        )
        si = drain_inst.ins.sync_info
        if si is not None and len(si.on_wait) > 1:
            waits = list(si.on_wait)
            updates = list(si.on_update)
            drain_inst.ins.sync_info = mybir.SyncInfo(
                on_wait=[waits[0]], on_update=[]
            )
            for i, w in enumerate(waits[1:]):
                extra = nc.sync.drain()
                ups = updates if i == len(waits) - 2 else []
                extra.ins.sync_info = mybir.SyncInfo(on_wait=[w], on_update=ups)
        nc.all_engine_barrier()
        assert self.sems is not None
        popped = nc._tile_sem_poison_stack.pop()
        assert popped is self._sem_poison
        nc.clear_and_free_semaphores(list(self.sems.allocated().values()))
        nc.all_engine_barrier()

    TileContext._drain_and_barrier = _patched
    TileContext._ant_drain_patched = True


def _patch_bir_wait_split():
    """walrus here rejects instructions with >1 sync wait; post-process the
    serialized BIR to hoist excess waits onto injected same-engine Drains."""
    import json
    import concourse.bass as bass

    if getattr(bass.Bass, "_ant_waitsplit_patched", False):
        return
    orig = bass.Bass.to_json_bytes

    def _split(doc):
        def fix_block(blk):
            insts = blk.get("instructions")
            if not isinstance(insts, list):
                return
            out = []
            for ins in insts:
                si = ins.get("sync_info")
                if si and isinstance(si, dict):
                    w = si.get("on_wait") or []
                    if len(w) > 1:
                        for j, wt in enumerate(w[:-1]):
                            out.append({
                                "name": f"{ins['name']}_sw{j}",
                                "engine": ins.get("engine", "SP"),
                                "opcode": "Drain",
                                "ins": [],
                                "outs": [],
                                "debug": ins.get("debug", 0),
                                "sync_info": {"on_wait": [wt], "on_update": []},
                            })
                        si["on_wait"] = [w[-1]]
                out.append(ins)
            blk["instructions"] = out

        def walk(o):
            if isinstance(o, dict):
                if "instructions" in o:
                    fix_block(o)
                for v in o.values():
                    walk(v)
            elif isinstance(o, list):
                for v in o:
                    walk(v)

        walk(doc)
        return doc

    def to_json_bytes(self, *a, **k):
        raw = orig(self, *a, **k)
        doc = json.loads(raw)
        return json.dumps(_split(doc)).encode()

    bass.Bass.to_json_bytes = to_json_bytes
    bass.Bass._ant_waitsplit_patched = True


def _row(h, half, d):
    # combined-layout row index for head h, half (0=content, 1=pos/sine), dim d
    return h * 64 + half * 32 + d


def _build_weights(inp):
    """Host-side weight shuffling into the merged concat layouts (bf16)."""
    w = {}
    s = SCALE

    def cat_w(content_w, pos_w):
        # -> [512, 256]: rows in combined layout; returns transposed [256, 512]
        m = np.zeros((512, C), np.float32)
        for h in range(H):
            if content_w is not None:
                m[_row(h, 0, 0) : _row(h, 0, 0) + 32] = content_w[
                    h * 32 : (h + 1) * 32
                ]
            if pos_w is not None:
                m[_row(h, 1, 0) : _row(h, 1, 0) + 32] = pos_w[h * 32 : (h + 1) * 32]
        return np.ascontiguousarray(m.T).astype(BF16)

    w["wq_q"] = cat_w(s * inp["W_qc"], None)
    w["wq_qp"] = cat_w(s * inp["W_qp"], None)
    w["wq_qs"] = cat_w(None, s * inp["W_qs"])
    w["wk_k"] = cat_w(inp["W_kc"], None)
    w["wk_kp"] = cat_w(inp["W_kp"], inp["W_kp"])

    bq = np.zeros((512,), np.float32)
    bk = np.zeros((512,), np.float32)
    for h in range(H):
        bq[_row(h, 0, 0) : _row(h, 0, 0) + 32] = s * (
            inp["b_qc"][h * 32 : (h + 1) * 32] + inp["b_qp"][h * 32 : (h + 1) * 32]
        )
        bq[_row(h, 1, 0) : _row(h, 1, 0) + 32] = s * inp["b_qs"][h * 32 : (h + 1) * 32]
        bk[_row(h, 0, 0) : _row(h, 0, 0) + 32] = (
            inp["b_kc"][h * 32 : (h + 1) * 32] + inp["b_kp"][h * 32 : (h + 1) * 32]
        )
        bk[_row(h, 1, 0) : _row(h, 1, 0) + 32] = inp["b_kp"][h * 32 : (h + 1) * 32]
    # [128, 4] per-partition bias columns, one per combined tile g
    w["bq"] = np.ascontiguousarray(bq.reshape(4, 128).T).astype(np.float32)
    w["bk"] = np.ascontiguousarray(bk.reshape(4, 128).T).astype(np.float32)

    # vext: [tok, 264] layout. Even heads (pair A): cols h*33 + [v(32)|ones];
    # odd heads (pair B): cols h*33 + [ones|v(32)], so the B denominator lands
    # at psum row 64 (a legal matmul operand base partition).
    wv = np.zeros((264, C), np.float32)
    bv = np.zeros((264,), np.float32)
    for h in range(H):
        off = h * 33 + (1 if h % 2 else 0)
        wv[off : off + 32] = inp["W_v"][h * 32 : (h + 1) * 32]
        bv[off : off + 32] = inp["b_v"][h * 32 : (h + 1) * 32]
        bv[h * 33 + (0 if h % 2 else 32)] = 1.0
    w["wv"] = np.ascontiguousarray(wv.T).astype(BF16)  # [256, 264]
    w["bias_v"] = np.broadcast_to(bv.astype(BF16), (128, 264)).copy()

    # o-proj rhs [128, 4*256]: pair g cols g*256+c; head 2g dims at rows 0:32
    # (attn rows 0:32, denom row 32 zeroed), head 2g+1 dims at rows 65:97
    # (attn rows 65:97, denom row 64 zeroed).
    wo = np.zeros((128, 4 * 256), np.float32)
    for g in range(4):
        blk = inp["W_o"][:, g * 64 : (g + 1) * 64]  # [256 out, 64 dims]
        wo[0:32, g * 256 : (g + 1) * 256] = blk[:, 0:32].T
        wo[65:97, g * 256 : (g + 1) * 256] = blk[:, 32:64].T
    w["wo"] = wo.astype(BF16)
    w["ones"] = np.ones((128, 128), BF16)
    return w


def _build_program():
    import concourse.bass as bass
    import concourse.mybir as mybir
    from concourse.tile import TileContext

    _patch_tile_tail_drain()
    _patch_bir_wait_split()
    f32 = mybir.dt.float32
    bf16 = mybir.dt.bfloat16
    AF = mybir.ActivationFunctionType

    nc = bass.Bass()

    dr = {}
    for nm in ("keyT", "kposT", "valT"):
        dr[nm] = nc.dram_tensor(nm, [BPC, 2, 128, HW], bf16, kind="ExternalInput")
    for nm in ("qT", "qposT", "qsineT"):
        dr[nm] = nc.dram_tensor(nm, [BPC, 2, 128, N_Q], bf16, kind="ExternalInput")
    dr["q_res"] = nc.dram_tensor("q_res", [N_Q, BPC, C], f32, kind="ExternalInput")
    for nm in ("wq_q", "wq_qp", "wq_qs", "wk_k", "wk_kp"):
        dr[nm] = nc.dram_tensor(nm, [2, 128, 512], bf16, kind="ExternalInput")
    dr["wv"] = nc.dram_tensor("wv", [2, 128, 264], bf16, kind="ExternalInput")
    dr["bias_v"] = nc.dram_tensor("bias_v", [128, 264], bf16, kind="ExternalInput")
    dr["bq"] = nc.dram_tensor("bq", [128, 4], f32, kind="ExternalInput")
    dr["bk"] = nc.dram_tensor("bk", [128, 4], f32, kind="ExternalInput")
    dr["wo"] = nc.dram_tensor("wo", [128, 1024], bf16, kind="ExternalInput")
    dr["ones"] = nc.dram_tensor("ones", [128, 128], bf16, kind="ExternalInput")
    out_d = nc.dram_tensor("out", [N_Q, BPC, C], f32, kind="ExternalOutput")

    from contextlib import ExitStack

    KCH = 4  # dma chunks per [128, HW] input half
    CHW = HW // KCH

    with TileContext(nc) as tc, ExitStack() as st:
        constp = st.enter_context(tc.tile_pool(name="const", bufs=1))
        kinp = st.enter_context(tc.tile_pool(name="kin", bufs=2))
        vinp = st.enter_context(tc.tile_pool(name="vin", bufs=2))
        qinp = st.enter_context(tc.tile_pool(name="qin", bufs=2))
        qcombp = st.enter_context(tc.tile_pool(name="qcomb", bufs=5))
        kcombp = st.enter_context(tc.tile_pool(name="kcomb", bufs=3))
        vextp = st.enter_context(tc.tile_pool(name="vext", bufs=2))
        expp = st.enter_context(tc.tile_pool(name="expb", bufs=4))
        attnp = st.enter_context(tc.tile_pool(name="attn", bufs=5))
        recipp = st.enter_context(tc.tile_pool(name="recip", bufs=2))
        rcrepp = st.enter_context(tc.tile_pool(name="rcrep", bufs=2))
        residp = st.enter_context(tc.tile_pool(name="resid", bufs=3))
        outsp = st.enter_context(tc.tile_pool(name="outs", bufs=3))
        projps = st.enter_context(tc.tile_pool(name="proj_ps", bufs=2, space="PSUM"))
        scoreps = st.enter_context(tc.tile_pool(name="score_ps", bufs=2, space="PSUM"))
        avps = st.enter_context(tc.tile_pool(name="av_ps", bufs=2, space="PSUM"))
        if True:
            # ---- load constants (weights first: qcomb needs them)
            cw = {}
            for nm in ("wq_q", "wq_qp", "wq_qs", "wk_k", "wk_kp"):
                cw[nm] = [constp.tile([128, 512], bf16, tag=f"{nm}{k}", name=f"{nm}{k}") for k in range(2)]
                for k in range(2):
                    nc.sync.dma_start(out=cw[nm][k][:], in_=dr[nm][k])
            cw["wv"] = [constp.tile([128, 264], bf16, tag=f"wv{k}", name=f"wv{k}") for k in range(2)]
            for k in range(2):
                nc.sync.dma_start(out=cw["wv"][k][:], in_=dr["wv"][k])
            for nm, shp, dt in (
                ("bias_v", [128, 264], bf16),
                ("bq", [128, 4], f32),
                ("bk", [128, 4], f32),
                ("wo", [128, 1024], bf16),
                ("ones", [128, 128], bf16),
            ):
                cw[nm] = constp.tile(shp, dt, tag=nm, name=nm)
                nc.sync.dma_start(out=cw[nm][:], in_=dr[nm][:])

            for b in range(BPC):
                # ---- q-side inputs first (small; unblocks qcomb)
                qin = {}
                for nm in ("qT", "qposT", "qsineT"):
                    qin[nm] = [qinp.tile([128, N_Q], bf16, tag=f"qin{nm}{k}", name=f"qin{nm}{k}") for k in range(2)]
                    for k in range(2):
                        nc.sync.dma_start(out=qin[nm][k][:], in_=dr[nm][b, k])
                # ---- k/v inputs in column chunks (key/kpos/val rotating)
                kin = {}
                for nm in ("keyT", "kposT"):
                    kin[nm] = [kinp.tile([128, HW], bf16, tag=f"kin{nm}{k}", name=f"kin{nm}{k}") for k in range(2)]
                vin = [vinp.tile([128, HW], bf16, tag=f"vin{k}", name=f"vin{k}") for k in range(2)]
                for c in range(KCH):
                    cs = slice(c * CHW, (c + 1) * CHW)
                    for nm in ("keyT", "kposT"):
                        for k in range(2):
                            nc.sync.dma_start(
                                out=kin[nm][k][:, cs], in_=dr[nm][b, k, :, cs]
                            )
                    for k in range(2):
                        nc.sync.dma_start(out=vin[k][:, cs], in_=dr["valT"][b, k, :, cs])

                # ---- qcombT [4][128, 300]
                qcomb = []
                for g in range(4):
                    ps = projps.tile([128, 512], f32, tag="pps", name="pps")
                    mm = 0
                    for wnm, xnm in (
                        ("wq_q", "qT"),
                        ("wq_qp", "qposT"),
                        ("wq_qs", "qsineT"),
                    ):
                        for k in range(2):
                            nc.tensor.matmul(
                                ps[:, 0:N_Q],
                                lhsT=cw[wnm][k][:, g * 128 : (g + 1) * 128],
                                rhs=qin[xnm][k][:, 0:N_Q],
                                start=(mm == 0),
                                stop=(mm == 5),
                                skip_group_check=True,
                            )
                            mm += 1
                    qt = qcombp.tile([128, N_Q], bf16)
                    nc.vector.tensor_scalar_add(
                        out=qt[:], in0=ps[:, 0:N_Q], scalar1=cw["bq"][:, g : g + 1]
                    )
                    qcomb.append(qt)

                # vext slab for this batch; tiles computed inside g==0's loop
                vext = vextp.tile([128, 32 * 264], bf16)

                # ---- attention per head pair
                attn_tiles = []
                for g in range(4):
                    av = avps.tile([128, 512], f32, tag="av", name="av")
                    for l4 in range(8):
                        kps = projps.tile([128, 512], f32, tag="pps", name="pps")
                        mm = 0
                        for wnm, xnm in (("wk_k", "keyT"), ("wk_kp", "kposT")):
                            for k in range(2):
                                nc.tensor.matmul(
                                    kps[:, 0:512],
                                    lhsT=cw[wnm][k][:, g * 128 : (g + 1) * 128],
                                    rhs=kin[xnm][k][:, l4 * 512 : (l4 + 1) * 512],
                                    start=(mm == 0),
                                    stop=(mm == 3),
                                    skip_group_check=True,
                                )
                                mm += 1
                        kcomb = kcombp.tile([128, 512], bf16)
                        nc.vector.tensor_scalar_add(
                            out=kcomb[:],
                            in0=kps[:, 0:512],
                            scalar1=cw["bk"][:, g : g + 1],
                        )
                        for sub in range(4):
                            lt = l4 * 4 + sub
                            if g == 0:
                                # vext tile for token block lt (needed by AV
                                # of every pair at this lt; pair 0 first)
                                vps = projps.tile([128, 512], f32, tag="pps", name="pps")
                                for k in range(2):
                                    nc.tensor.matmul(
                                        vps[:, 0:264],
                                        lhsT=vin[k][:, lt * 128 : (lt + 1) * 128],
                                        rhs=cw["wv"][k][:],
                                        start=(k == 0),
                                        stop=(k == 1),
                                        skip_group_check=True,
                                    )
                                nc.vector.tensor_add(
                                    out=vext[:, lt * 264 : (lt + 1) * 264],
                                    in0=vps[:, 0:264],
                                    in1=cw["bias_v"][:],
                                )
                            sc = scoreps.tile([128, 1024], f32, tag="sc", name="sc")
                            nc.tensor.matmul(
                                sc[:, 0:N_Q],
                                lhsT=kcomb[0:64, sub * 128 : (sub + 1) * 128],
                                rhs=qcomb[g][0:64, :],
                                start=True,
                                stop=True,
                                skip_group_check=True,
                            )
                            nc.tensor.matmul(
                                sc[:, 512 : 512 + N_Q],
                                lhsT=kcomb[64:128, sub * 128 : (sub + 1) * 128],
                                rhs=qcomb[g][64:128, :],
                                start=True,
                                stop=True,
                                skip_group_check=True,
                            )
                            ex = expp.tile([128, 2 * N_Q], bf16)
                            sc_v = sc[:].rearrange("p (two n) -> p two n", two=2)
                            ex_v = ex[:].rearrange("p (two n) -> p two n", two=2)
                            nc.scalar.activation(
                                out=ex_v[:, :, 0:N_Q],
                                in_=sc_v[:, :, 0:N_Q],
                                func=AF.Exp,
                            )
                            # AV: col-tiled pair, one psum bank
                            # A: rows 0:33 (out 0:32 + denom 32)
                            # B: rows 64:97 (out 64:96 + denom 96)
                            vA = vext[:, lt * 264 + (2 * g) * 33 : lt * 264 + (2 * g) * 33 + 33]
                            vB = vext[:, lt * 264 + (2 * g + 1) * 33 : lt * 264 + (2 * g + 1) * 33 + 33]
                            nc.tensor.matmul(
                                av[0:33, 0:N_Q],
                                lhsT=vA,
                                rhs=ex[:, 0:N_Q],
                                start=(lt == 0),
                                stop=(lt == 31),
                                skip_group_check=True,
                                tile_position=(0, 0),
                            )
                            nc.tensor.matmul(
                                av[64:97, 0:N_Q],
                                lhsT=vB,
                                rhs=ex[:, N_Q : 2 * N_Q],
                                start=(lt == 0),
                                stop=(lt == 31),
                                skip_group_check=True,
                                tile_position=(0, 64),
                            )
                    # ---- normalize pair g -> attnT bf16 rows {0:33, 64:97}
                    # A: values rows 0:32, denom row 32; B: denom row 64,
                    # values rows 65:97 (wo has zeros at rows 32 and 64).
                    rc = recipp.tile([128, N_Q], bf16)
                    with nc.allow_low_precision(reason="bf16 recip for PE replicate"):
                        nc.vector.reciprocal(
                            out=rc[32:33, 0:N_Q], in_=av[32:33, 0:N_Q]
                        )
                        nc.vector.reciprocal(
                            out=rc[64:65, 0:N_Q], in_=av[64:65, 0:N_Q]
                        )
                    # two psum banks so the two row-tiled replicates may
                    # overlap (same-bank concurrent row tiles are illegal)
                    rpA = projps.tile([128, 512], f32, tag="pps", name="pps")
                    rpB = projps.tile([128, 512], f32, tag="pps", name="pps")
                    nc.tensor.matmul(
                        rpA[0:64, 0:N_Q],
                        lhsT=cw["ones"][32:33, 0:64],
                        rhs=rc[32:33, 0:N_Q],
                        start=True,
                        stop=True,
                        skip_group_check=True,
                    )
                    nc.tensor.matmul(
                        rpB[64:128, 0:N_Q],
                        lhsT=cw["ones"][64:65, 0:64],
                        rhs=rc[64:65, 0:N_Q],
                        start=True,
                        stop=True,
                        skip_group_check=True,
                    )
                    rcrep = rcrepp.tile([128, N_Q], f32, tag="rcrep", name="rcrep")
                    nc.vector.tensor_copy(out=rcrep[0:64], in_=rpA[0:64, 0:N_Q])
                    nc.vector.tensor_copy(out=rcrep[64:128], in_=rpB[64:128, 0:N_Q])
                    at = attnp.tile([128, N_Q], bf16)
                    nc.vector.tensor_mul(
                        out=at[0:33, :],
                        in0=av[0:33, 0:N_Q],
                        in1=rcrep[0:33, :],
                    )
                    nc.vector.tensor_mul(
                        out=at[64:97, :],
                        in0=av[64:97, 0:N_Q],
                        in1=rcrep[64:97, :],
                    )
                    attn_tiles.append(at)

                # ---- output projection + residual (b_o folded into q_res)
                for n0, nsz in ((0, 128), (128, 128), (256, 44)):
                    psA = projps.tile([128, 512], f32, tag="pps", name="pps")
                    psB = projps.tile([128, 512], f32, tag="pps", name="pps")
                    for g in range(4):
                        nc.tensor.matmul(
                            psA[0:nsz, 0:256],
                            lhsT=attn_tiles[g][0:33, n0 : n0 + nsz],
                            rhs=cw["wo"][0:33, g * 256 : (g + 1) * 256],
                            start=(g == 0),
                            stop=(g == 3),
                            skip_group_check=True,
                        )
                    for g in range(4):
                        nc.tensor.matmul(
                            psB[0:nsz, 0:256],
                            lhsT=attn_tiles[g][64:97, n0 : n0 + nsz],
                            rhs=cw["wo"][64:97, g * 256 : (g + 1) * 256],
                            start=(g == 0),
                            stop=(g == 3),
                            skip_group_check=True,
                        )
                    res = residp.tile([128, 256], f32)
                    nc.sync.dma_start(
                        out=res[0:nsz], in_=dr["q_res"][n0 : n0 + nsz, b]
                    )
                    ot = outsp.tile([128, 256], f32)
                    nc.vector.tensor_add(
                        out=ot[0:nsz], in0=psA[0:nsz, 0:256], in1=res[0:nsz]
                    )
                    ot2 = outsp.tile([128, 256], f32)
                    nc.vector.tensor_add(
                        out=ot2[0:nsz], in0=psB[0:nsz, 0:256], in1=ot[0:nsz]
                    )
                    nc.sync.dma_start(out=out_d[n0 : n0 + nsz, b], in_=ot2[0:nsz])

    return nc


def _get_program():
    if "nc" not in _COMPILED:
        _COMPILED["nc"] = _build_program()
    return _COMPILED["nc"]


def _host_inputs(inputs, core):
    """Per-core in_map: slice batches, cast bf16, pre-transpose."""
    bs = slice(core * BPC, (core + 1) * BPC)
    m = dict(_COMPILED["weights"])

    def t_in(x):  # [T, bpc, C] -> [bpc, 2, 128, T] bf16
        a = np.ascontiguousarray(np.transpose(x[:, bs, :], (1, 2, 0))).astype(BF16)
        return a.reshape(BPC, 2, 128, x.shape[0])

    m["keyT"] = t_in(inputs["key"])
    m["kposT"] = t_in(inputs["key_pos"])
    m["valT"] = t_in(inputs["value"])
    m["qT"] = t_in(inputs["query"])
    m["qposT"] = t_in(inputs["query_pos"])
    m["qsineT"] = t_in(inputs["query_sine_embed"])
    # residual with b_o pre-added (saves the bias matmul on device)
    m["q_res"] = (
        inputs["query"][:, bs, :].astype(np.float32)
        + inputs["b_o"].astype(np.float32)[None, None, :]
    )
    return m


def kernel(**inputs):
    from concourse.bass_utils import run_bass_kernel_spmd

    inputs = {k: np.asarray(v) for k, v in inputs.items()}
    _COMPILED["weights"] = {
        k: v for k, v in _build_weights(inputs).items()
    }
    nc = _get_program()
    in_maps = [_host_inputs(inputs, i) for i in range(NCORES)]
    res = run_bass_kernel_spmd(nc, in_maps, core_ids=list(range(NCORES)))
    outs = [res.results[i]["out"] for i in range(NCORES)]
    return np.concatenate(outs, axis=1).astype(np.float32)


if __name__ == "__main__":
    sys.path.insert(0, "/root/problem")
    import reference

    inp = {k: np.asarray(v) for k, v in reference.setup_inputs().items()}
    exp = np.asarray(reference.reference(**inp))
    act = kernel(**inp)
    err = np.linalg.norm(act - exp) / np.linalg.norm(exp)
    print("rel l2 err:", err)
    print("max abs err:", np.max(np.abs(act - exp)))


# revision 11
# speedup vs baseline: 1.0739x; 1.0739x over previous
"""Trainium2 Bass kernel for ConditionalCrossAttention (DETR-style).

Shapes (hardcoded): query/query_pos/query_sine_embed [300,16,256],
key/key_pos/value [4096,16,256], 7 linear projections 256x256, H=8 heads,
per-head dim 32, concat content|positional -> head dim 64, softmax over
L=4096, output projection + residual.

Sharding: batch B=16 across 8 cores (2 per core), no collectives.

Device-side design per core (b in {0,1}, head pairs g in {0..3}):
  - Host pre-transposes inputs to [C, tokens] bf16 and builds merged
    "concat layout" projection weights (scale folded into q side).
  - DMA order: weights, q-side, then key/kpos/val in 1024-column chunks
    so the first kproj starts ~9us in; batch b+1 inputs stream during
    batch b compute.
  - qcombT [4][128,300], kcombT per (g,l4) [128,512] via PE matmuls,
    bias applied during DVE eviction.
  - vext pre-pass interleaved into pair-0's attention loop: value
    projection in layout [tok, 264]: per head h 33 cols = [v_h(32)|ones].
  - scores^T [l,n]: two K=64 matmuls per pair (row tiles (0,0)/(64,0),
    concurrent), Exp on ScalarE over both banks -> bf16 SBUF.
  - AV: two col-tiled matmuls per (pair, l-tile): M=33 ([v|ones]) at
    tile positions (0,0) and (0,64) -> ONE psum bank per pair
    (A rows 0:33, B rows 64:97; denominators at rows 32/96).
    avps bufs=2 so pair g+1 accumulates while pair g normalizes.
  - normalize: DVE reciprocal straight from PSUM rows 32/96, PE
    ones-matmul replicates recip across partitions, DVE multiply ->
    attnT bf16 rows {0:32, 64:96}.
  - o-proj: K=32 chunks from attn rows 0:32 (psum A) and 64:96 (psum B)
    in parallel row tiles, b_o folded into the fp32 residual host-side,
    two DVE adds, DMA out fp32.
"""

import sys

for _p in ("/opt/trn_rl_repo", "/root/.axon_site/_ro/trn_rl_repo"):
    if _p not in sys.path:
        sys.path.insert(0, _p)

import numpy as np
import ml_dtypes

BF16 = ml_dtypes.bfloat16

N_Q = 300
HW = 4096
B = 16
C = 256
H = 8
DH = 32
NCORES = 8
BPC = B // NCORES  # batches per core
SCALE = (2 * DH) ** (-0.5)

_COMPILED = {}


def _patch_tile_tail_drain():
    """walrus in this container rejects >1-2 sync waits on the Tile tail
    Drain; split them across a chain of single-wait drains."""
    import concourse.mybir as mybir
    from concourse.tile import TileContext
    from concourse.vector_clock import ScopedClock

    if getattr(TileContext, "_ant_drain_patched", False):
        return

    def _patched(self, tick_clock, wait_clock):
        nc = self.nc
        drain_inst = nc.sync.drain()
        wait_clock.add_sem_waits(
            drain_inst.ins, ScopedClock({None: tick_clock.global_clock})
        )
        si = drain_inst.ins.sync_info
        if si is not None and len(si.on_wait) > 1:
            waits = list(si.on_wait)
            updates = list(si.on_update)
            drain_inst.ins.sync_info = mybir.SyncInfo(
                on_wait=[waits[0]], on_update=[]
            )
            for i, w in enumerate(waits[1:]):
                extra = nc.sync.drain()
                ups = updates if i == len(waits) - 2 else []
                extra.ins.sync_info = mybir.SyncInfo(on_wait=[w], on_update=ups)
        nc.all_engine_barrier()
        assert self.sems is not None
        popped = nc._tile_sem_poison_stack.pop()
        assert popped is self._sem_poison
        nc.clear_and_free_semaphores(list(self.sems.allocated().values()))
        nc.all_engine_barrier()

    TileContext._drain_and_barrier = _patched
    TileContext._ant_drain_patched = True


def _patch_bir_wait_split():
    """walrus here rejects instructions with >1 sync wait; post-process the
    serialized BIR to hoist excess waits onto injected same-engine Drains."""
    import json
    import concourse.bass as bass

    if getattr(bass.Bass, "_ant_waitsplit_patched", False):
        return
    orig = bass.Bass.to_json_bytes

    def _split(doc):
        def fix_block(blk):
            insts = blk.get("instructions")
            if not isinstance(insts, list):
                return
            out = []
            for ins in insts:
                si = ins.get("sync_info")
                if si and isinstance(si, dict):
                    w = si.get("on_wait") or []
                    if len(w) > 1:
                        for j, wt in enumerate(w[:-1]):
                            out.append({
                                "name": f"{ins['name']}_sw{j}",
                                "engine": ins.get("engine", "SP"),
                                "opcode": "Drain",
                                "ins": [],
                                "outs": [],
                                "debug": ins.get("debug", 0),
                                "sync_info": {"on_wait": [wt], "on_update": []},
                            })
                        si["on_wait"] = [w[-1]]
                out.append(ins)
            blk["instructions"] = out

        def walk(o):
            if isinstance(o, dict):
                if "instructions" in o:
                    fix_block(o)
                for v in o.values():
                    walk(v)
            elif isinstance(o, list):
                for v in o:
                    walk(v)

        walk(doc)
        return doc

    def to_json_bytes(self, *a, **k):
        raw = orig(self, *a, **k)
        doc = json.loads(raw)
        return json.dumps(_split(doc)).encode()

    bass.Bass.to_json_bytes = to_json_bytes
    bass.Bass._ant_waitsplit_patched = True


def _row(h, half, d):
    # combined-layout row index for head h, half (0=content, 1=pos/sine), dim d
    return h * 64 + half * 32 + d


def _build_weights(inp):
    """Host-side weight shuffling into the merged concat layouts (bf16)."""
    w = {}
    s = SCALE

    def cat_w(content_w, pos_w):
        # -> [512, 256]: rows in combined layout; returns transposed [256, 512]
        m = np.zeros((512, C), np.float32)
        for h in range(H):
            if content_w is not None:
                m[_row(h, 0, 0) : _row(h, 0, 0) + 32] = content_w[
                    h * 32 : (h + 1) * 32
                ]
            if pos_w is not None:
                m[_row(h, 1, 0) : _row(h, 1, 0) + 32] = pos_w[h * 32 : (h + 1) * 32]
        return np.ascontiguousarray(m.T).astype(BF16)

    w["wq_q"] = cat_w(s * inp["W_qc"], None)
    w["wq_qp"] = cat_w(s * inp["W_qp"], None)
    w["wq_qs"] = cat_w(None, s * inp["W_qs"])
    w["wk_k"] = cat_w(inp["W_kc"], None)
    w["wk_kp"] = cat_w(inp["W_kp"], inp["W_kp"])

    bq = np.zeros((512,), np.float32)
    bk = np.zeros((512,), np.float32)
    for h in range(H):
        bq[_row(h, 0, 0) : _row(h, 0, 0) + 32] = s * (
            inp["b_qc"][h * 32 : (h + 1) * 32] + inp["b_qp"][h * 32 : (h + 1) * 32]
        )
        bq[_row(h, 1, 0) : _row(h, 1, 0) + 32] = s * inp["b_qs"][h * 32 : (h + 1) * 32]
        bk[_row(h, 0, 0) : _row(h, 0, 0) + 32] = (
            inp["b_kc"][h * 32 : (h + 1) * 32] + inp["b_kp"][h * 32 : (h + 1) * 32]
        )
        bk[_row(h, 1, 0) : _row(h, 1, 0) + 32] = inp["b_kp"][h * 32 : (h + 1) * 32]
    # [128, 4] per-partition bias columns, one per combined tile g
    w["bq"] = np.ascontiguousarray(bq.reshape(4, 128).T).astype(np.float32)
    w["bk"] = np.ascontiguousarray(bk.reshape(4, 128).T).astype(np.float32)

    # vext: [tok, 264] layout. Even heads (pair A): cols h*33 + [v(32)|ones];
    # odd heads (pair B): cols h*33 + [ones|v(32)], so the B denominator lands
    # at psum row 64 (a legal matmul operand base partition).
    wv = np.zeros((264, C), np.float32)
    bv = np.zeros((264,), np.float32)
    for h in range(H):
        off = h * 33 + (1 if h % 2 else 0)
        wv[off : off + 32] = inp["W_v"][h * 32 : (h + 1) * 32]
        bv[off : off + 32] = inp["b_v"][h * 32 : (h + 1) * 32]
        bv[h * 33 + (0 if h % 2 else 32)] = 1.0
    w["wv"] = np.ascontiguousarray(wv.T).astype(BF16)  # [256, 264]
    w["bias_v"] = np.broadcast_to(bv.astype(BF16), (128, 264)).copy()

    # o-proj rhs [128, 4*256]: pair g cols g*256+c; head 2g dims at rows 0:32
    # (attn rows 0:32, denom row 32 zeroed), head 2g+1 dims at rows 65:97
    # (attn rows 65:97, denom row 64 zeroed).
    wo = np.zeros((128, 4 * 256), np.float32)
    for g in range(4):
        blk = inp["W_o"][:, g * 64 : (g + 1) * 64]  # [256 out, 64 dims]
        wo[0:32, g * 256 : (g + 1) * 256] = blk[:, 0:32].T
        wo[65:97, g * 256 : (g + 1) * 256] = blk[:, 32:64].T
    w["wo"] = wo.astype(BF16)
    w["ones"] = np.ones((128, 128), BF16)
    # fp32 ones rows for the recip replicate (fp32 rhs needs fp32 lhsT)
    w["ones_f32"] = np.ones((128, 64), np.float32)
    return w


def _build_program():
    import concourse.bass as bass
    import concourse.mybir as mybir
    from concourse.tile import TileContext

    _patch_tile_tail_drain()
    _patch_bir_wait_split()
    f32 = mybir.dt.float32
    bf16 = mybir.dt.bfloat16
    AF = mybir.ActivationFunctionType

    nc = bass.Bass()

    dr = {}
    for nm in ("keyT", "kposT", "valT"):
        dr[nm] = nc.dram_tensor(nm, [BPC, 2, 128, HW], bf16, kind="ExternalInput")
    for nm in ("qT", "qposT", "qsineT"):
        dr[nm] = nc.dram_tensor(nm, [BPC, 2, 128, N_Q], bf16, kind="ExternalInput")
    dr["q_res"] = nc.dram_tensor("q_res", [N_Q, BPC, C], f32, kind="ExternalInput")
    for nm in ("wq_q", "wq_qp", "wq_qs", "wk_k", "wk_kp"):
        dr[nm] = nc.dram_tensor(nm, [2, 128, 512], bf16, kind="ExternalInput")
    dr["wv"] = nc.dram_tensor("wv", [2, 128, 264], bf16, kind="ExternalInput")
    dr["bias_v"] = nc.dram_tensor("bias_v", [128, 264], bf16, kind="ExternalInput")
    dr["bq"] = nc.dram_tensor("bq", [128, 4], f32, kind="ExternalInput")
    dr["bk"] = nc.dram_tensor("bk", [128, 4], f32, kind="ExternalInput")
    dr["wo"] = nc.dram_tensor("wo", [128, 1024], bf16, kind="ExternalInput")
    dr["ones"] = nc.dram_tensor("ones", [128, 128], bf16, kind="ExternalInput")
    dr["ones_f32"] = nc.dram_tensor("ones_f32", [128, 64], f32, kind="ExternalInput")
    out_d = nc.dram_tensor("out", [N_Q, BPC, C], f32, kind="ExternalOutput")

    from contextlib import ExitStack

    KCH = 4  # dma chunks per [128, HW] input half
    CHW = HW // KCH

    with TileContext(nc) as tc, ExitStack() as st:
        constp = st.enter_context(tc.tile_pool(name="const", bufs=1))
        kinp = st.enter_context(tc.tile_pool(name="kin", bufs=2))
        vinp = st.enter_context(tc.tile_pool(name="vin", bufs=2))
        qinp = st.enter_context(tc.tile_pool(name="qin", bufs=2))
        qcombp = st.enter_context(tc.tile_pool(name="qcomb", bufs=5))
        kcombp = st.enter_context(tc.tile_pool(name="kcomb", bufs=3))
        vextp = st.enter_context(tc.tile_pool(name="vext", bufs=2))
        expp = st.enter_context(tc.tile_pool(name="expb", bufs=4))
        attnp = st.enter_context(tc.tile_pool(name="attn", bufs=5))
        recipp = st.enter_context(tc.tile_pool(name="recip", bufs=2))
        rcrepp = st.enter_context(tc.tile_pool(name="rcrep", bufs=2))
        residp = st.enter_context(tc.tile_pool(name="resid", bufs=3))
        outsp = st.enter_context(tc.tile_pool(name="outs", bufs=3))
        projps = st.enter_context(tc.tile_pool(name="proj_ps", bufs=2, space="PSUM"))
        scoreps = st.enter_context(tc.tile_pool(name="score_ps", bufs=2, space="PSUM"))
        avps = st.enter_context(tc.tile_pool(name="av_ps", bufs=2, space="PSUM"))
        if True:
            # ---- load constants (weights first: qcomb needs them)
            cw = {}
            for nm in ("wq_q", "wq_qp", "wq_qs", "wk_k", "wk_kp"):
                cw[nm] = [constp.tile([128, 512], bf16, tag=f"{nm}{k}", name=f"{nm}{k}") for k in range(2)]
                for k in range(2):
                    nc.sync.dma_start(out=cw[nm][k][:], in_=dr[nm][k])
            cw["wv"] = [constp.tile([128, 264], bf16, tag=f"wv{k}", name=f"wv{k}") for k in range(2)]
            for k in range(2):
                nc.sync.dma_start(out=cw["wv"][k][:], in_=dr["wv"][k])
            for nm, shp, dt in (
                ("bias_v", [128, 264], bf16),
                ("bq", [128, 4], f32),
                ("bk", [128, 4], f32),
                ("wo", [128, 1024], bf16),
                ("ones", [128, 128], bf16),
                ("ones_f32", [128, 64], f32),
            ):
                cw[nm] = constp.tile(shp, dt, tag=nm, name=nm)
                nc.sync.dma_start(out=cw[nm][:], in_=dr[nm][:])

            for b in range(BPC):
                # ---- q-side inputs first (small; unblocks qcomb)
                qin = {}
                for nm in ("qT", "qposT", "qsineT"):
                    qin[nm] = [qinp.tile([128, N_Q], bf16, tag=f"qin{nm}{k}", name=f"qin{nm}{k}") for k in range(2)]
                    for k in range(2):
                        nc.sync.dma_start(out=qin[nm][k][:], in_=dr[nm][b, k])
                # ---- k/v inputs in column chunks (key/kpos/val rotating)
                kin = {}
                for nm in ("keyT", "kposT"):
                    kin[nm] = [kinp.tile([128, HW], bf16, tag=f"kin{nm}{k}", name=f"kin{nm}{k}") for k in range(2)]
                vin = [vinp.tile([128, HW], bf16, tag=f"vin{k}", name=f"vin{k}") for k in range(2)]
                for c in range(KCH):
                    cs = slice(c * CHW, (c + 1) * CHW)
                    for nm in ("keyT", "kposT"):
                        for k in range(2):
                            nc.sync.dma_start(
                                out=kin[nm][k][:, cs], in_=dr[nm][b, k, :, cs]
                            )
                    for k in range(2):
                        nc.sync.dma_start(out=vin[k][:, cs], in_=dr["valT"][b, k, :, cs])

                # ---- qcombT [4][128, 300]
                qcomb = []
                for g in range(4):
                    ps = projps.tile([128, 512], f32, tag="pps", name="pps")
                    mm = 0
                    for wnm, xnm in (
                        ("wq_q", "qT"),
                        ("wq_qp", "qposT"),
                        ("wq_qs", "qsineT"),
                    ):
                        for k in range(2):
                            nc.tensor.matmul(
                                ps[:, 0:N_Q],
                                lhsT=cw[wnm][k][:, g * 128 : (g + 1) * 128],
                                rhs=qin[xnm][k][:, 0:N_Q],
                                start=(mm == 0),
                                stop=(mm == 5),
                                skip_group_check=True,
                            )
                            mm += 1
                    qt = qcombp.tile([128, N_Q], bf16)
                    nc.vector.tensor_scalar_add(
                        out=qt[:], in0=ps[:, 0:N_Q], scalar1=cw["bq"][:, g : g + 1]
                    )
                    qcomb.append(qt)

                # vext slab for this batch; tiles computed inside g==0's loop
                vext = vextp.tile([128, 32 * 264], bf16)

                # ---- attention per head pair
                attn_tiles = []

                def emit_normalize(av):
                    # normalize pair -> attnT bf16 rows {0:33, 64:97}
                    # A: values rows 0:32, denom row 32; B: denom row 64,
                    # values rows 65:97 (wo has zeros at rows 32 and 64).
                    # one instruction covers both denom rows (32=A, 64=B);
                    # other lanes compute recip of AV values/garbage, free
                    rcf = recipp.tile([128, N_Q], f32)
                    nc.vector.reciprocal(
                        out=rcf[0:65, 0:N_Q], in_=av[0:65, 0:N_Q]
                    )
                    # two psum banks so the two row-tiled replicates may
                    # overlap (same-bank concurrent row tiles are illegal)
                    rpA = projps.tile([128, 512], f32, tag="pps", name="pps")
                    rpB = projps.tile([128, 512], f32, tag="pps", name="pps")
                    nc.tensor.matmul(
                        rpA[0:64, 0:N_Q],
                        lhsT=cw["ones_f32"][32:33, 0:64],
                        rhs=rcf[32:33, 0:N_Q],
                        start=True,
                        stop=True,
                        skip_group_check=True,
                    )
                    nc.tensor.matmul(
                        rpB[64:128, 0:N_Q],
                        lhsT=cw["ones_f32"][64:65, 0:64],
                        rhs=rcf[64:65, 0:N_Q],
                        start=True,
                        stop=True,
                        skip_group_check=True,
                    )
                    rcrep = rcrepp.tile([128, N_Q], f32, tag="rcrep", name="rcrep")
                    nc.vector.tensor_copy(out=rcrep[0:64], in_=rpA[0:64, 0:N_Q])
                    nc.vector.tensor_copy(out=rcrep[64:128], in_=rpB[64:128, 0:N_Q])
                    at = attnp.tile([128, N_Q], bf16)
                    nc.vector.tensor_mul(
                        out=at[0:33, :],
                        in0=av[0:33, 0:N_Q],
                        in1=rcrep[0:33, :],
                    )
                    nc.vector.tensor_mul(
                        out=at[64:97, :],
                        in0=av[64:97, 0:N_Q],
                        in1=rcrep[64:97, :],
                    )
                    attn_tiles.append(at)

                pending_av = None  # previous pair's accumulator, to normalize
                for g in range(4):
                    av = avps.tile([128, 512], f32, tag="av", name="av")
                    for l4 in range(8):
                        # --- 128x128-mode block: kproj (+ vext when g==0)
                        kps = projps.tile([128, 512], f32, tag="pps", name="pps")
                        mm = 0
                        for wnm, xnm in (("wk_k", "keyT"), ("wk_kp", "kposT")):
                            for k in range(2):
                                nc.tensor.matmul(
                                    kps[:, 0:512],
                                    lhsT=cw[wnm][k][:, g * 128 : (g + 1) * 128],
                                    rhs=kin[xnm][k][:, l4 * 512 : (l4 + 1) * 512],
                                    start=(mm == 0),
                                    stop=(mm == 3),
                                    skip_group_check=True,
                                )
                                mm += 1
                        kcomb = kcombp.tile([128, 512], bf16)
                        nc.vector.tensor_scalar_add(
                            out=kcomb[:],
                            in0=kps[:, 0:512],
                            scalar1=cw["bk"][:, g : g + 1],
                        )
                        if g == 0:
                            for sub in range(4):
                                lt = l4 * 4 + sub
                                vps = projps.tile([128, 512], f32, tag="pps", name="pps")
                                for k in range(2):
                                    nc.tensor.matmul(
                                        vps[:, 0:264],
                                        lhsT=vin[k][:, lt * 128 : (lt + 1) * 128],
                                        rhs=cw["wv"][k][:],
                                        start=(k == 0),
                                        stop=(k == 1),
                                        skip_group_check=True,
                                    )
                                nc.vector.tensor_add(
                                    out=vext[:, lt * 264 : (lt + 1) * 264],
                                    in0=vps[:, 0:264],
                                    in1=cw["bias_v"][:],
                                )
                        # --- two sub-pairs: scores x2 (row tiles), exp x2,
                        # AV x2 (col tiles) — grouped by PE tiling mode
                        for half in range(2):
                            lts = (l4 * 4 + 2 * half, l4 * 4 + 2 * half + 1)
                            scs, exs = [], []
                            for lt in lts:
                                sub = lt % 4
                                sc = scoreps.tile([128, 1024], f32, tag="sc", name="sc")
                                nc.tensor.matmul(
                                    sc[:, 0:N_Q],
                                    lhsT=kcomb[0:64, sub * 128 : (sub + 1) * 128],
                                    rhs=qcomb[g][0:64, :],
                                    start=True,
                                    stop=True,
                                    skip_group_check=True,
                                )
                                nc.tensor.matmul(
                                    sc[:, 512 : 512 + N_Q],
                                    lhsT=kcomb[64:128, sub * 128 : (sub + 1) * 128],
                                    rhs=qcomb[g][64:128, :],
                                    start=True,
                                    stop=True,
                                    skip_group_check=True,
                                )
                                scs.append(sc)
                            for sc in scs:
                                ex = expp.tile([128, 2 * N_Q], bf16)
                                sc_v = sc[:].rearrange("p (two n) -> p two n", two=2)
                                ex_v = ex[:].rearrange("p (two n) -> p two n", two=2)
                                nc.scalar.activation(
                                    out=ex_v[:, :, 0:N_Q],
                                    in_=sc_v[:, :, 0:N_Q],
                                    func=AF.Exp,
                                )
                                exs.append(ex)
                            if pending_av is not None:
                                # previous pair's normalize, off the PE
                                # critical path (recip ran during kproj)
                                emit_normalize(pending_av)
                                pending_av = None
                            for lt, ex in zip(lts, exs):
                                vA = vext[:, lt * 264 + (2 * g) * 33 : lt * 264 + (2 * g) * 33 + 33]
                                vB = vext[:, lt * 264 + (2 * g + 1) * 33 : lt * 264 + (2 * g + 1) * 33 + 33]
                                nc.tensor.matmul(
                                    av[0:33, 0:N_Q],
                                    lhsT=vA,
                                    rhs=ex[:, 0:N_Q],
                                    start=(lt == 0),
                                    stop=(lt == 31),
                                    skip_group_check=True,
                                    tile_position=(0, 0),
                                )
                                nc.tensor.matmul(
                                    av[64:97, 0:N_Q],
                                    lhsT=vB,
                                    rhs=ex[:, N_Q : 2 * N_Q],
                                    start=(lt == 0),
                                    stop=(lt == 31),
                                    skip_group_check=True,
                                    tile_position=(0, 64),
                                )
                    pending_av = av
                emit_normalize(pending_av)
                pending_av = None

                # ---- output projection + residual (b_o folded into q_res)
                for n0, nsz in ((0, 128), (128, 128), (256, 44)):
                    psA = projps.tile([128, 512], f32, tag="pps", name="pps")
                    psB = projps.tile([128, 512], f32, tag="pps", name="pps")
                    for g in range(4):
                        nc.tensor.matmul(
                            psA[0:nsz, 0:256],
                            lhsT=attn_tiles[g][0:33, n0 : n0 + nsz],
                            rhs=cw["wo"][0:33, g * 256 : (g + 1) * 256],
                            start=(g == 0),
                            stop=(g == 3),
                            skip_group_check=True,
                        )
                    for g in range(4):
                        nc.tensor.matmul(
                            psB[0:nsz, 0:256],
                            lhsT=attn_tiles[g][64:97, n0 : n0 + nsz],
                            rhs=cw["wo"][64:97, g * 256 : (g + 1) * 256],
                            start=(g == 0),
                            stop=(g == 3),
                            skip_group_check=True,
                        )
                    res = residp.tile([128, 256], f32)
                    nc.sync.dma_start(
                        out=res[0:nsz], in_=dr["q_res"][n0 : n0 + nsz, b]
                    )
                    ot = outsp.tile([128, 256], f32)
                    nc.vector.tensor_add(
                        out=ot[0:nsz], in0=psA[0:nsz, 0:256], in1=res[0:nsz]
                    )
                    ot2 = outsp.tile([128, 256], f32)
                    nc.vector.tensor_add(
                        out=ot2[0:nsz], in0=psB[0:nsz, 0:256], in1=ot[0:nsz]
                    )
                    nc.sync.dma_start(out=out_d[n0 : n0 + nsz, b], in_=ot2[0:nsz])

    return nc


def _get_program():
    if "nc" not in _COMPILED:
        _COMPILED["nc"] = _build_program()
    return _COMPILED["nc"]


def _host_inputs(inputs, core):
    """Per-core in_map: slice batches, cast bf16, pre-transpose."""
    bs = slice(core * BPC, (core + 1) * BPC)
    m = dict(_COMPILED["weights"])

    def t_in(x):  # [T, bpc, C] -> [bpc, 2, 128, T] bf16
        a = np.ascontiguousarray(np.transpose(x[:, bs, :], (1, 2, 0))).astype(BF16)
        return a.reshape(BPC, 2, 128, x.shape[0])

    m["keyT"] = t_in(inputs["key"])
    m["kposT"] = t_in(inputs["key_pos"])
    m["valT"] = t_in(inputs["value"])
    m["qT"] = t_in(inputs["query"])
    m["qposT"] = t_in(inputs["query_pos"])
    m["qsineT"] = t_in(inputs["query_sine_embed"])
    # residual with b_o pre-added (saves the bias matmul on device)
    m["q_res"] = (
        inputs["query"][:, bs, :].astype(np.float32)
        + inputs["b_o"].astype(np.float32)[None, None, :]
    )
    return m


def kernel(**inputs):
    from concourse.bass_utils import run_bass_kernel_spmd

    inputs = {k: np.asarray(v) for k, v in inputs.items()}
    _COMPILED["weights"] = {
        k: v for k, v in _build_weights(inputs).items()
    }
    nc = _get_program()
    in_maps = [_host_inputs(inputs, i) for i in range(NCORES)]
    res = run_bass_kernel_spmd(nc, in_maps, core_ids=list(range(NCORES)))
    outs = [res.results[i]["out"] for i in range(NCORES)]
    return np.concatenate(outs, axis=1).astype(np.float32)


if __name__ == "__main__":
    sys.path.insert(0, "/root/problem")
    import reference

    inp = {k: np.asarray(v) for k, v in reference.setup_inputs().items()}
    exp = np.asarray(reference.reference(**inp))
    act = kernel(**inp)
    err = np.linalg.norm(act - exp) / np.linalg.norm(exp)
    print("rel l2 err:", err)
    print("max abs err:", np.max(np.abs(act - exp)))
